# revision 20
# baseline (speedup 1.0000x reference)
"""Group-quantized linear (fake int4 per-group dequant) GEMV on 8 Trainium2 cores.

Reference computation (all fp32):
    qw = round_half_even(clip(W, -8, 7))            # W in [-8, 7) so clip is identity
    out = (qw.reshape(O, 64, 128) * scales[:, :, None]).reshape(O, O) @ x

Sharding: column-parallel — each core owns a 1024-row slice of W/scales,
x replicated, outputs concatenated (per the tensor-parallel hint).

Key ideas:
- qw is a small-integer tensor (ints in [-8, 7]) which fp8e4m3 represents
  EXACTLY; quantization is computed bit-exactly on the host (same fp32
  round-half-even as the reference) and shipped as fp8, cutting HBM weight
  traffic 4x (32 MiB -> 8 MiB/core).  HW exec is then HBM-streaming-bound.
- The GEMV is restructured so the WEIGHTS are the PE's moving operand and a
  block-diagonal x is the stationary operand; with perf_mode=DoubleRow the
  fp8 array virtualizes to 128x256 and streams TWO weight channels per
  column-cycle, keeping the TensorEngine well under the DMA rate:

    per step u (32 steps), output chunk oc (2):
      lhsT [Ki=128, Ko=2, M=128] = xblk: row (g,cb), col (g',h) ->
           delta(g,g') * x8_h[g*128 + cb*64 + 2u + ko]     (h = Dekker hi/lo)
      rhs  [Ki=128, Ko=2, N=512] = qw[o, g*128 + cb*64 + 2u + ko]  (fp8)
      psum_oc[(g,h), o] += sum_{cb,ko} x8_h[...] * qw[o, ...]

  x is split x = hi + lo with both parts e4m3 (Dekker), recovering ~8
  mantissa bits; with exact int weights this lands at ~2e-3 rel err.
- All weight chunks get their own SBUF tiles (8 MiB fits SBUF) so every
  DMA is issued upfront and the ring streams continuously; a short burst
  of dummy matmuls warms the PE clock gate (HAM) during the pre-stream gap.
- Epilogue: z[(g,h), o] = psum * scalesT (scales duplicated over h on the
  host), then out[o] = ones[128].T @ z — partition reduction on the PE.
"""

import numpy as np
import ml_dtypes

IN_DIM = 8192
OUT_DIM = 8192
NG = 64  # quantization groups (128 channels each)
N_CORES = 8
PER_OUT = OUT_DIM // N_CORES  # 1024
P = 128
U = 32  # steps: each covers 4 channels/group = (cb in {0,1}) x (ko in {0,1})
OC_W = 512  # output chunk width (one PSUM bank)

_cache = {}

UCHUNKS = [2, 4, 4, 4, 4, 4, 4, 4, 2]  # u-steps per weight DMA (sum 32)


def _split_multi_waits(nc):
    """walrus in this container accepts only ONE sync-wait per instruction;
    Tile's tail drain carries one per producer proc. Hoist extras onto
    same-engine NoOps placed immediately before — identical semantics for an
    in-order sequencer."""
    import concourse.mybir as mybir

    uid = 0
    for f in nc.m.functions:
        for blk in f.blocks:
            insts = blk.instructions
            if not any(
                i.sync_info is not None
                and i.sync_info.on_wait
                and len(i.sync_info.on_wait) > 1
                for i in insts
            ):
                continue
            new_insts = []
            for inst in insts:
                si = inst.sync_info
                if si is not None and si.on_wait and len(si.on_wait) > 1:
                    waits = list(si.on_wait)
                    for w in waits[:-1]:
                        uid += 1
                        new_insts.append(
                            mybir.InstNoOp(
                                name=f"I-waitsplit-{uid}",
                                engine=inst.engine,
                                ins=[],
                                outs=[],
                                sync_info=mybir.SyncInfo(on_wait=[w], on_update=[]),
                            )
                        )
                    inst.sync_info = mybir.SyncInfo(
                        on_wait=[waits[-1]], on_update=si.on_update
                    )
                new_insts.append(inst)
            blk.instructions = new_insts
    return nc


def build_nc(split_waits=True, n_warmup=8, uchunks=None):
    import concourse.bass as bass
    import concourse.mybir as mybir
    import concourse.tile as tile

    f32 = mybir.dt.float32
    bf16 = mybir.dt.bfloat16
    f8 = mybir.dt.float8e4
    mult = mybir.AluOpType.mult
    DR = mybir.MatmulPerfMode.DoubleRow

    if uchunks is None:
        uchunks = UCHUNKS
    assert sum(uchunks) == U

    nc = bass.Bass()
    # [(g,cb), u, ko, o] fp8: qw[o, g*128 + cb*64 + 2u + ko]
    wq = nc.dram_tensor("wq", [P, U, 2, PER_OUT], f8, kind="ExternalInput")
    # block-diagonal Dekker-split x [128, 32 u, 2 ko, 128 (h,g')] fp8
    xb_d = nc.dram_tensor("xb", [P, U, 2, P], f8, kind="ExternalInput")
    # scales transposed + duplicated over h: [128 (h,g), 2 oc, 512 o']
    st_d = nc.dram_tensor("st", [P, 2, OC_W], f32, kind="ExternalInput")
    out_d = nc.dram_tensor("out", [PER_OUT], f32, kind="ExternalOutput")

    with tile.TileContext(nc) as tc:
        with (
            tc.tile_pool(name="singles", bufs=1) as singles,
            tc.tile_pool(name="psum", bufs=1, space="PSUM") as psum,
        ):
            # ---- weight DMAs: every chunk gets its own SBUF tile (8 MiB
            # total fits SBUF) so ALL transfers are issued upfront and the
            # ring streams continuously with no buffer-reuse waits.
            ones = singles.tile([P, 1], bf16)
            nc.gpsimd.memset(ones, 1.0)
            # All weights on the sync HWDGE ring.  Each chunk is issued as
            # TWO transfers — partitions [0:112] and [112:128] — so the
            # second transfer's engine spray restarts at engine 0 and the
            # structurally-slow last SDMA engine (observed ~18% slower on
            # this fabric, a growing straggler that gated every chunk's
            # completion sem) carries a reduced share.
            PSPLIT = 112
            wtiles = []
            u0 = 0
            for ci, clen in enumerate(uchunks):
                wt_ = singles.tile([P, clen, 2, PER_OUT], f8, name=f"w{ci}")
                nc.sync.dma_start(
                    wt_[0:PSPLIT], wq.ap()[0:PSPLIT, u0 : u0 + clen, :, :]
                )
                nc.sync.dma_start(
                    wt_[PSPLIT:P], wq.ap()[PSPLIT:P, u0 : u0 + clen, :, :]
                )
                wtiles.append((u0, clen, wt_))
                u0 += clen

            # ---- aux loads ride the scalar HWDGE ring, in parallel
            xblk = singles.tile([P, U, 2, P], f8)
            nc.scalar.dma_start(xblk[:, 0:2, :, :], xb_d.ap()[:, 0:2, :, :])
            nc.scalar.dma_start(xblk[:, 2:U, :, :], xb_d.ap()[:, 2:U, :, :])
            st = singles.tile([P, 2, OC_W], f32)
            nc.scalar.dma_start(st, st_d.ap())

            # ---- PE warm-up: dummy matmuls with no DMA dependency so the
            # HAM clock gate reaches K=8/8 before the real stream begins.
            wm_ps = psum.tile([1, OC_W], f32, tag="warm")
            for _ in range(n_warmup):
                nc.tensor.matmul(
                    wm_ps,
                    lhsT=ones[:, 0:1],
                    rhs=ones.broadcast_to([P, OC_W]),
                    start=True,
                    stop=True,
                )

            # ---- main: DoubleRow fp8 — 2 interleaved weights per PE cell
            accs = [
                psum.tile([P, OC_W], f32, tag="acc0", name="acc0"),
                psum.tile([P, OC_W], f32, tag="acc1", name="acc1"),
            ]
            for u0, clen, wt_ in wtiles:
                for ul in range(clen):
                    u = u0 + ul
                    for oc in range(2):
                        nc.tensor.matmul(
                            accs[oc],
                            lhsT=xblk[:, u, :, :],
                            rhs=wt_[:, ul, :, oc * OC_W : (oc + 1) * OC_W],
                            start=(u == 0),
                            stop=(u == U - 1),
                            perf_mode=DR,
                        )

            # ---- epilogue: z = y * scalesT (h-duplicated), then the
            # partition reduction out[o] = sum_{g,h} z[(g,h), o] via a
            # ones-vector matmul; psum -> sbuf copies split across DVE and
            # ScalarE so they overlap.
            out_sb = singles.tile([1, PER_OUT], f32)
            for oc in range(2):
                z = singles.tile([P, OC_W], bf16, name=f"z{oc}")
                nc.vector.tensor_tensor(z, accs[oc], st[:, oc, :], mult)
                ops = psum.tile([1, OC_W], f32, tag=f"ored{oc}")
                nc.tensor.matmul(
                    ops, lhsT=ones, rhs=z, start=True, stop=True
                )
                dst = out_sb[:, oc * OC_W : (oc + 1) * OC_W]
                if oc == 0:
                    nc.vector.tensor_copy(out=dst, in_=ops)
                else:
                    nc.scalar.copy(out=dst, in_=ops)
            nc.sync.dma_start(out_d.rearrange("(a o) -> a o", a=1), out_sb)

    return _split_multi_waits(nc) if split_waits else nc


def _prep_inputs(x, weights, scales):
    """Host-side shard + layout. Quantization here is bit-exact vs the
    reference (same fp32 round-half-even; ints in [-8,7] are exact in fp8)."""
    f8t = ml_dtypes.float8_e4m3
    x = np.ascontiguousarray(np.asarray(x, dtype=np.float32))
    weights = np.asarray(weights, dtype=np.float32)
    scales = np.asarray(scales, dtype=np.float32)

    # Dekker split of x into two e4m3 parts: x ~ hi + lo
    xhi = x.astype(f8t).astype(np.float32)
    xlo = (x - xhi).astype(f8t).astype(np.float32)
    # channel index k = g*128 + cb*64 + 2u + ko  ->  [p=(g,cb), u, ko]
    xs = np.stack([xhi, xlo])  # [h, 8192]
    xs = xs.reshape(2, NG, 2, U, 2)  # [h, g, cb, u, ko]
    msk = np.repeat(np.eye(NG, dtype=np.float32), 2, axis=0)  # [(g,cb), g']
    # xb[(g,cb), u, ko, h*64+g'] = delta(g,g') * xs[h, g, cb, u, ko]
    vals = xs.transpose(1, 2, 3, 4, 0)  # [g, cb, u, ko, h]
    vals = vals.reshape(P, U, 2, 2)  # [(g,cb), u, ko, h]
    xb = vals[:, :, :, :, None] * msk.reshape(P, 1, 1, 1, NG)  # [..., h, g']
    xb = np.ascontiguousarray(xb.reshape(P, U, 2, P).astype(f8t))

    in_maps = []
    for c in range(N_CORES):
        sl = slice(c * PER_OUT, (c + 1) * PER_OUT)
        qw = np.rint(np.clip(weights[sl], -8.0, 7.0))
        # [o, (g, cb, u, ko)] -> [(g, cb), u, ko, o]
        wqa = (
            qw.reshape(PER_OUT, NG, 2, U, 2)
            .transpose(1, 2, 3, 4, 0)
            .reshape(P, U, 2, PER_OUT)
        )
        wqa = np.ascontiguousarray(wqa).astype(f8t)
        s_t = scales[sl].T  # [g, o] = [64, 1024]
        # st[h*64+g, oc, o'] = scales[oc*512+o', g]  (same for h=0,1)
        s_go = np.stack([s_t[:, :OC_W], s_t[:, OC_W:]], axis=1)  # [g, oc, o']
        st = np.ascontiguousarray(np.concatenate([s_go, s_go], axis=0))  # [128, 2, 512]
        in_maps.append({"wq": wqa, "xb": xb, "st": st})
    return in_maps


def kernel(x, weights, scales):
    from concourse import bass_utils

    if "nc" not in _cache:
        _cache["nc"] = build_nc()
    nc = _cache["nc"]

    in_maps = _prep_inputs(x, weights, scales)
    res = bass_utils.run_bass_kernel_spmd(nc, in_maps, core_ids=list(range(N_CORES)))
    return np.concatenate([res.results[c]["out"] for c in range(N_CORES)]).astype(
        np.float32
    )


# revision 21
# speedup vs baseline: 1.3987x; 1.3987x over previous
"""Group-quantized linear (fake int4 per-group dequant) GEMV on 8 Trainium2 cores.

Reference computation (all fp32):
    qw = round_half_even(clip(W, -8, 7))            # W in [-8, 7) so clip is identity
    out = (qw.reshape(O, 64, 128) * scales[:, :, None]).reshape(O, O) @ x

Sharding: column-parallel — each core owns a 1024-row slice of W/scales,
x replicated, outputs concatenated (per the tensor-parallel hint).

Key ideas:
- qw is a small-integer tensor (ints in [-8, 7]) which fp8e4m3 represents
  EXACTLY; quantization is computed bit-exactly on the host (same fp32
  round-half-even as the reference) and shipped as fp8, cutting HBM weight
  traffic 4x (32 MiB -> 8 MiB/core).  HW exec is then HBM-streaming-bound.
- The GEMV is restructured so the WEIGHTS are the PE's moving operand and a
  block-diagonal x is the stationary operand; with perf_mode=DoubleRow the
  fp8 array virtualizes to 128x256 and streams TWO weight channels per
  column-cycle, keeping the TensorEngine well under the DMA rate:

    per step u (32 steps), output chunk oc (2):
      lhsT [Ki=128, Ko=2, M=128] = xblk: row (g,cb), col (g',h) ->
           delta(g,g') * x8_h[g*128 + cb*64 + 2u + ko]     (h = Dekker hi/lo)
      rhs  [Ki=128, Ko=2, N=512] = qw[o, g*128 + cb*64 + 2u + ko]  (fp8)
      psum_oc[(g,h), o] += sum_{cb,ko} x8_h[...] * qw[o, ...]

  x is split x = hi + lo with both parts e4m3 (Dekker), recovering ~8
  mantissa bits; with exact int weights this lands at ~2e-3 rel err.
- All weight chunks get their own SBUF tiles (8 MiB fits SBUF) so every
  DMA is issued upfront and the ring streams continuously; a short burst
  of dummy matmuls warms the PE clock gate (HAM) during the pre-stream gap.
- Epilogue: z[(g,h), o] = psum * scalesT (scales duplicated over h on the
  host), then out[o] = ones[128].T @ z — partition reduction on the PE.
"""

import numpy as np
import ml_dtypes

IN_DIM = 8192
OUT_DIM = 8192
NG = 64  # quantization groups (128 channels each)
N_CORES = 8
PER_OUT = OUT_DIM // N_CORES  # 1024
P = 128
U = 32  # steps: each covers 4 channels/group = (cb in {0,1}) x (ko in {0,1})
OC_W = 512  # output chunk width (one PSUM bank)

_cache = {}

UCHUNKS = [2, 4, 4, 4, 4, 4, 4, 4, 2]  # u-steps per weight DMA (sum 32)


def _split_multi_waits(nc):
    """walrus in this container accepts only ONE sync-wait per instruction;
    Tile's tail drain carries one per producer proc. Hoist extras onto
    same-engine NoOps placed immediately before — identical semantics for an
    in-order sequencer."""
    import concourse.mybir as mybir

    uid = 0
    for f in nc.m.functions:
        for blk in f.blocks:
            insts = blk.instructions
            if not any(
                i.sync_info is not None
                and i.sync_info.on_wait
                and len(i.sync_info.on_wait) > 1
                for i in insts
            ):
                continue
            new_insts = []
            for inst in insts:
                si = inst.sync_info
                if si is not None and si.on_wait and len(si.on_wait) > 1:
                    waits = list(si.on_wait)
                    for w in waits[:-1]:
                        uid += 1
                        new_insts.append(
                            mybir.InstNoOp(
                                name=f"I-waitsplit-{uid}",
                                engine=inst.engine,
                                ins=[],
                                outs=[],
                                sync_info=mybir.SyncInfo(on_wait=[w], on_update=[]),
                            )
                        )
                    inst.sync_info = mybir.SyncInfo(
                        on_wait=[waits[-1]], on_update=si.on_update
                    )
                new_insts.append(inst)
            blk.instructions = new_insts
    return nc


def build_nc(split_waits=True, n_warmup=8, uchunks=None):
    import concourse.bass as bass
    import concourse.mybir as mybir
    import concourse.tile as tile

    f32 = mybir.dt.float32
    bf16 = mybir.dt.bfloat16
    f8 = mybir.dt.float8e4
    mult = mybir.AluOpType.mult
    DR = mybir.MatmulPerfMode.DoubleRow

    if uchunks is None:
        uchunks = UCHUNKS
    assert sum(uchunks) == U

    nc = bass.Bass()
    # [(g,cb), u, ko, o] fp8: qw[o, g*128 + cb*64 + 2u + ko]
    wq = nc.dram_tensor("wq", [P, U, 2, PER_OUT], f8, kind="ExternalInput")
    # block-diagonal Dekker-split x [128, 32 u, 2 ko, 128 (h,g')] fp8
    xb_d = nc.dram_tensor("xb", [P, U, 2, P], f8, kind="ExternalInput")
    # scales transposed + duplicated over h: [128 (h,g), 2 oc, 512 o']
    st_d = nc.dram_tensor("st", [P, 2, OC_W], f32, kind="ExternalInput")
    out_d = nc.dram_tensor("out", [PER_OUT], f32, kind="ExternalOutput")

    with tile.TileContext(nc) as tc:
        with (
            tc.tile_pool(name="singles", bufs=1) as singles,
            tc.tile_pool(name="psum", bufs=1, space="PSUM") as psum,
        ):
            # ---- weight DMAs: every chunk gets its own SBUF tile (8 MiB
            # total fits SBUF) so ALL transfers are issued upfront and the
            # ring streams continuously with no buffer-reuse waits.
            ones = singles.tile([P, 1], bf16)
            nc.gpsimd.memset(ones, 1.0)
            # All weights stream on the sync HWDGE ring.
            wtiles = []
            u0 = 0
            for ci, clen in enumerate(uchunks):
                wt_ = singles.tile([P, clen, 2, PER_OUT], f8, name=f"w{ci}")
                nc.sync.dma_start(wt_, wq.ap()[:, u0 : u0 + clen, :, :])
                wtiles.append((u0, clen, wt_))
                u0 += clen

            # ---- aux loads ride the scalar HWDGE ring, in parallel
            xblk = singles.tile([P, U, 2, P], f8)
            nc.scalar.dma_start(xblk[:, 0:2, :, :], xb_d.ap()[:, 0:2, :, :])
            nc.scalar.dma_start(xblk[:, 2:U, :, :], xb_d.ap()[:, 2:U, :, :])
            st = singles.tile([P, 2, OC_W], f32)
            nc.scalar.dma_start(st, st_d.ap())

            # ---- PE warm-up: dummy matmuls with no DMA dependency so the
            # HAM clock gate reaches K=8/8 before the real stream begins.
            wm_ps = psum.tile([1, OC_W], f32, tag="warm")
            for _ in range(n_warmup):
                nc.tensor.matmul(
                    wm_ps,
                    lhsT=ones[:, 0:1],
                    rhs=ones.broadcast_to([P, OC_W]),
                    start=True,
                    stop=True,
                )

            # ---- main: DoubleRow fp8 — 2 interleaved weights per PE cell
            accs = [
                psum.tile([P, OC_W], f32, tag="acc0", name="acc0"),
                psum.tile([P, OC_W], f32, tag="acc1", name="acc1"),
            ]
            for u0, clen, wt_ in wtiles:
                for ul in range(clen):
                    u = u0 + ul
                    for oc in range(2):
                        nc.tensor.matmul(
                            accs[oc],
                            lhsT=xblk[:, u, :, :],
                            rhs=wt_[:, ul, :, oc * OC_W : (oc + 1) * OC_W],
                            start=(u == 0),
                            stop=(u == U - 1),
                            perf_mode=DR,
                        )

            # ---- epilogue: z = y * scalesT (h-duplicated), then the
            # partition reduction out[o] = sum_{g,h} z[(g,h), o] via a
            # ones-vector matmul; psum -> sbuf copies split across DVE and
            # ScalarE so they overlap.
            out_sb = singles.tile([1, PER_OUT], f32)
            for oc in range(2):
                z = singles.tile([P, OC_W], bf16, name=f"z{oc}")
                nc.vector.tensor_tensor(z, accs[oc], st[:, oc, :], mult)
                ops = psum.tile([1, OC_W], f32, tag=f"ored{oc}")
                nc.tensor.matmul(
                    ops, lhsT=ones, rhs=z, start=True, stop=True
                )
                dst = out_sb[:, oc * OC_W : (oc + 1) * OC_W]
                if oc == 0:
                    nc.vector.tensor_copy(out=dst, in_=ops)
                else:
                    nc.scalar.copy(out=dst, in_=ops)
            nc.sync.dma_start(out_d.rearrange("(a o) -> a o", a=1), out_sb)

    return _split_multi_waits(nc) if split_waits else nc


def _prep_inputs(x, weights, scales):
    """Host-side shard + layout. Quantization here is bit-exact vs the
    reference (same fp32 round-half-even; ints in [-8,7] are exact in fp8)."""
    f8t = ml_dtypes.float8_e4m3
    x = np.ascontiguousarray(np.asarray(x, dtype=np.float32))
    weights = np.asarray(weights, dtype=np.float32)
    scales = np.asarray(scales, dtype=np.float32)

    # Dekker split of x into two e4m3 parts: x ~ hi + lo
    xhi = x.astype(f8t).astype(np.float32)
    xlo = (x - xhi).astype(f8t).astype(np.float32)
    # channel index k = g*128 + cb*64 + 2u + ko  ->  [p=(g,cb), u, ko]
    xs = np.stack([xhi, xlo])  # [h, 8192]
    xs = xs.reshape(2, NG, 2, U, 2)  # [h, g, cb, u, ko]
    msk = np.repeat(np.eye(NG, dtype=np.float32), 2, axis=0)  # [(g,cb), g']
    # xb[(g,cb), u, ko, h*64+g'] = delta(g,g') * xs[h, g, cb, u, ko]
    vals = xs.transpose(1, 2, 3, 4, 0)  # [g, cb, u, ko, h]
    vals = vals.reshape(P, U, 2, 2)  # [(g,cb), u, ko, h]
    xb = vals[:, :, :, :, None] * msk.reshape(P, 1, 1, 1, NG)  # [..., h, g']
    xb = np.ascontiguousarray(xb.reshape(P, U, 2, P).astype(f8t))

    in_maps = []
    for c in range(N_CORES):
        sl = slice(c * PER_OUT, (c + 1) * PER_OUT)
        qw = np.rint(np.clip(weights[sl], -8.0, 7.0))
        # [o, (g, cb, u, ko)] -> [(g, cb), u, ko, o]
        wqa = (
            qw.reshape(PER_OUT, NG, 2, U, 2)
            .transpose(1, 2, 3, 4, 0)
            .reshape(P, U, 2, PER_OUT)
        )
        wqa = np.ascontiguousarray(wqa).astype(f8t)
        s_t = scales[sl].T  # [g, o] = [64, 1024]
        # st[h*64+g, oc, o'] = scales[oc*512+o', g]  (same for h=0,1)
        s_go = np.stack([s_t[:, :OC_W], s_t[:, OC_W:]], axis=1)  # [g, oc, o']
        st = np.ascontiguousarray(np.concatenate([s_go, s_go], axis=0))  # [128, 2, 512]
        in_maps.append({"wq": wqa, "xb": xb, "st": st})
    return in_maps


def kernel(x, weights, scales):
    from concourse import bass_utils

    if "nc" not in _cache:
        _cache["nc"] = build_nc()
    nc = _cache["nc"]

    in_maps = _prep_inputs(x, weights, scales)
    res = bass_utils.run_bass_kernel_spmd(nc, in_maps, core_ids=list(range(N_CORES)))
    return np.concatenate([res.results[c]["out"] for c in range(N_CORES)]).astype(
        np.float32
    )
